# revision 14
# baseline (speedup 1.0000x reference)
"""Trainium2 Bass kernel for nn_Detector_73804718014475 (moe_routing).

Strategy: pure data-parallel over batch B=16 across 8 NeuronCores (B_loc=2).
Per core, for each of the 24 experts (l):
  - load h[b,l] [400,1024] fp32, cast to bf16 during SWDGE DMA,
  - transpose to hT [d,t] layout via HWDGE xbar DMA-transpose (bf16),
  - GEMM1 (bf16, fp32 PSUM): x1T[e1,t] = We1[l]^T @ hT, ReLU+bias -> bf16,
  - GEMM2 (bf16): x2T[e2,t] = We2[l]^T @ x1T, +bias -> fp32,
  - attention score via PE (fp32r), softmax on ACT/DVE,
  - pooled[e] = sum_t x2T[e,t]*softmaxw[t] via DVE tensor_tensor_reduce.
Router / fusion / LayerNorm / classifier are tiny and computed on-chip in a
transposed layout; 4 outputs are written per core and concatenated on host.
"""

import numpy as np
import ml_dtypes

B, L, T, D = 16, 24, 400, 1024
E = D // 2            # 512
RIN = E * L           # 12288
RH = 128
NCLS = 2
TEMP = 1.75
ALPHA = 0.325
N_CORES = 8
BL = B // N_CORES     # 2 batch elements per core

TF = 128              # t tile
NT_FULL = T // TF     # 3 full t-chunks
T_TAIL = T - NT_FULL * TF   # 16
EC = E // 128         # 4 chunks of e
DC = D // 128         # 8 chunks of d

_PROGRAM = None


def _build_program():
    from contextlib import ExitStack
    import concourse.bass as bass
    import concourse.tile as tile
    import concourse.mybir as mybir
    from concourse import bacc
    from concourse.masks import make_identity

    dt = mybir.dt
    AF = mybir.ActivationFunctionType
    ALU = mybir.AluOpType
    AX = mybir.AxisListType

    nc = bacc.Bacc("TRN2", target_bir_lowering=False, debug=False)

    # ---------------- DRAM I/O ----------------
    hx = nc.dram_tensor("hx", [BL, L, T, D], dt.float32, kind="ExternalInput").ap()
    noiset = nc.dram_tensor("noiset", [L, BL], dt.float32, kind="ExternalInput").ap()
    we1 = nc.dram_tensor("we1", [L, D, E], dt.bfloat16, kind="ExternalInput").ap()
    we2 = nc.dram_tensor("we2", [L, E, E], dt.bfloat16, kind="ExternalInput").ap()
    # vsc[l] = We2[l] @ wa (host-precomputed): score = x2@wa + const_l with the
    # const cancelling in softmax, so score can be computed from x1 directly.
    vsc = nc.dram_tensor("vsc", [L, E], dt.bfloat16, kind="ExternalInput").ap()
    wr1 = nc.dram_tensor("wr1", [RIN, RH], dt.bfloat16, kind="ExternalInput").ap()
    wn = nc.dram_tensor("wn", [RIN, L], dt.bfloat16, kind="ExternalInput").ap()
    wr2 = nc.dram_tensor("wr2", [RH, L], dt.bfloat16, kind="ExternalInput").ap()
    wc1 = nc.dram_tensor("wc1", [E, 256], dt.bfloat16, kind="ExternalInput").ap()
    wc2 = nc.dram_tensor("wc2", [256, NCLS], dt.bfloat16, kind="ExternalInput").ap()
    be1t = nc.dram_tensor("be1t", [128, L * EC], dt.float32, kind="ExternalInput").ap()
    be2t = nc.dram_tensor("be2t", [128, L * EC], dt.float32, kind="ExternalInput").ap()
    br1c = nc.dram_tensor("br1c", [RH, 1], dt.float32, kind="ExternalInput").ap()
    br2c = nc.dram_tensor("br2c", [L, 1], dt.float32, kind="ExternalInput").ap()
    bnc = nc.dram_tensor("bnc", [L, 1], dt.float32, kind="ExternalInput").ap()
    bc1t = nc.dram_tensor("bc1t", [128, 2], dt.float32, kind="ExternalInput").ap()
    bc2c = nc.dram_tensor("bc2c", [NCLS, 1], dt.float32, kind="ExternalInput").ap()
    lng = nc.dram_tensor("lng", [1, E], dt.float32, kind="ExternalInput").ap()
    lnb = nc.dram_tensor("lnb", [1, E], dt.float32, kind="ExternalInput").ap()

    o_logits = nc.dram_tensor("o_logits", [BL, NCLS], dt.float32, kind="ExternalOutput").ap()
    o_rw = nc.dram_tensor("o_rw", [BL, L], dt.float32, kind="ExternalOutput").ap()
    o_fn = nc.dram_tensor("o_fn", [BL, E], dt.float32, kind="ExternalOutput").ap()
    o_pooled = nc.dram_tensor("o_pooled", [BL, L, E], dt.float32, kind="ExternalOutput").ap()

    with tile.TileContext(nc) as tc, ExitStack() as ctx:
        # ---------------- pools ----------------
        singles = ctx.enter_context(tc.tile_pool(name="singles", bufs=1))
        wpool = ctx.enter_context(tc.tile_pool(name="wpool", bufs=2))
        hpool = ctx.enter_context(tc.tile_pool(name="hpool", bufs=2))
        htpool = ctx.enter_context(tc.tile_pool(name="htpool", bufs=2))
        xpool = ctx.enter_context(tc.tile_pool(name="xpool", bufs=2))
        smalls = ctx.enter_context(tc.tile_pool(name="smalls", bufs=4))
        epi = ctx.enter_context(tc.tile_pool(name="epi", bufs=1))
        ps_big = ctx.enter_context(tc.tile_pool(name="ps_big", bufs=2, space="PSUM"))
        ps_sm = ctx.enter_context(tc.tile_pool(name="ps_sm", bufs=4, space="PSUM"))

        # ---------------- constants / weights resident in SBUF ----------------
        id128 = singles.tile([128, 128], dt.float32, tag="id128")
        make_identity(nc, id128)

        vsc_sb = singles.tile([128, L, EC], dt.bfloat16, tag="vsc")
        nc.sync.dma_start(out=vsc_sb, in_=vsc.rearrange("l (c p) -> p l c", p=128))

        wr1_sb = singles.tile([128, L, EC, RH], dt.bfloat16, tag="wr1")
        nc.sync.dma_start(
            out=wr1_sb, in_=wr1.rearrange("(l c p) r -> p l c r", l=L, c=EC, p=128)
        )
        wn_sb = singles.tile([128, L, EC, L], dt.bfloat16, tag="wn")
        nc.sync.dma_start(
            out=wn_sb, in_=wn.rearrange("(l c p) r -> p l c r", l=L, c=EC, p=128)
        )
        wr2_sb = singles.tile([RH, L], dt.bfloat16, tag="wr2")
        nc.sync.dma_start(out=wr2_sb, in_=wr2)
        wc1_sb = singles.tile([128, EC, 256], dt.bfloat16, tag="wc1")
        nc.sync.dma_start(out=wc1_sb, in_=wc1.rearrange("(c p) f -> p c f", p=128))
        wc2_sb = singles.tile([128, 2, NCLS], dt.bfloat16, tag="wc2")
        nc.sync.dma_start(out=wc2_sb, in_=wc2.rearrange("(c p) f -> p c f", p=128))

        be1_sb = singles.tile([128, L * EC], dt.float32, tag="be1")
        nc.sync.dma_start(out=be1_sb, in_=be1t)
        be2_sb = singles.tile([128, L * EC], dt.float32, tag="be2")
        nc.sync.dma_start(out=be2_sb, in_=be2t)
        br1_sb = singles.tile([RH, 1], dt.float32, tag="br1")
        nc.sync.dma_start(out=br1_sb, in_=br1c)
        br2_sb = singles.tile([L, 1], dt.float32, tag="br2")
        nc.sync.dma_start(out=br2_sb, in_=br2c)
        bn_sb = singles.tile([L, 1], dt.float32, tag="bn")
        nc.sync.dma_start(out=bn_sb, in_=bnc)
        bc1_sb = singles.tile([128, 2], dt.float32, tag="bc1")
        nc.sync.dma_start(out=bc1_sb, in_=bc1t)
        bc2_sb = singles.tile([NCLS, 1], dt.float32, tag="bc2")
        nc.sync.dma_start(out=bc2_sb, in_=bc2c)
        noise_sb = singles.tile([L, BL], dt.float32, tag="noise")
        nc.sync.dma_start(out=noise_sb, in_=noiset)
        lng_sb = singles.tile([1, E], dt.float32, tag="lng")
        nc.sync.dma_start(out=lng_sb, in_=lng)
        lnb_sb = singles.tile([1, E], dt.float32, tag="lnb")
        nc.sync.dma_start(out=lnb_sb, in_=lnb)
        eps_sb = singles.tile([1, 1], dt.float32, tag="eps")
        nc.vector.memset(eps_sb, 1e-5)

        # pooled staging: [e_lo, l, e_hi, b] fp32 (router-input layout)
        pooled_sb = singles.tile([128, L, EC, BL], dt.float32, tag="pooled")
        # pooled transposed to natural [l, b, e] for output + fused matvec
        pooled_t = singles.tile([L, BL, E], dt.float32, tag="pooledt")

        # ---------------- main per-(l, b) loop ----------------
        for l in range(L):
            we1_sb = wpool.tile([128, DC, E], dt.bfloat16, tag="we1")
            nc.sync.dma_start(
                out=we1_sb, in_=we1[l].rearrange("(j p) e -> p j e", p=128)
            )
            we2_sb = wpool.tile([128, EC, E], dt.bfloat16, tag="we2")
            nc.sync.dma_start(
                out=we2_sb, in_=we2[l].rearrange("(c p) e -> p c e", p=128)
            )

            for b in range(BL):
                # load h[b,l] (cast fp32 -> bf16) as [t_lo, t_hi, d]
                hN = hpool.tile([128, NT_FULL + 1, D], dt.bfloat16, tag="hN")
                nc.gpsimd.dma_start(
                    out=hN[:, 0:NT_FULL, :],
                    in_=hx[b, l, 0 : NT_FULL * TF, :].rearrange(
                        "(c p) d -> p c d", p=128
                    ),
                )
                nc.gpsimd.dma_start(
                    out=hN[0:T_TAIL, NT_FULL, :], in_=hx[b, l, NT_FULL * TF : T, :]
                )

                # transpose to hT[d_lo, d_hi, t]
                hT = htpool.tile([128, DC, T], dt.bfloat16, tag="hT")
                for c in range(NT_FULL):
                    nc.sync.dma_start(
                        out=hT[:, :, c * TF : (c + 1) * TF],
                        in_=hN[:, c, :],
                        transpose=True,
                    )
                nc.sync.dma_start(
                    out=hT[:, :, NT_FULL * TF : T],
                    in_=hN[0:T_TAIL, NT_FULL, :],
                    transpose=True,
                )

                # GEMM1: x1T[e1,t] = We1^T @ hT  (bf16, accum fp32)
                x1t = xpool.tile([128, EC, T], dt.bfloat16, tag="x1t")
                for ec in range(EC):
                    px = ps_big.tile([128, T], dt.float32, tag="x1")
                    for j in range(DC):
                        nc.tensor.matmul(
                            px,
                            lhsT=we1_sb[:, j, ec * 128 : (ec + 1) * 128],
                            rhs=hT[:, j, :],
                            start=(j == 0),
                            stop=(j == DC - 1),
                        )
                    nc.scalar.activation(
                        out=x1t[:, ec, :],
                        in_=px,
                        func=AF.Relu,
                        bias=be1_sb[:, l * EC + ec : l * EC + ec + 1],
                    )

                # GEMM2: x2T[e2,t] = We2^T @ x1T (+bias via DVE) -> fp32
                x2t = xpool.tile([128, EC, T], dt.float32, tag="x2t")
                for fc in range(EC):
                    px = ps_big.tile([128, T], dt.float32, tag="x2")
                    for ec in range(EC):
                        nc.tensor.matmul(
                            px,
                            lhsT=we2_sb[:, ec, fc * 128 : (fc + 1) * 128],
                            rhs=x1t[:, ec, :],
                            start=(ec == 0),
                            stop=(ec == EC - 1),
                        )
                    nc.vector.tensor_scalar_add(
                        x2t[:, fc, :], px, be2_sb[:, l * EC + fc : l * EC + fc + 1]
                    )

                # score[t] = sum_e1 x1T[e1,t] * vsc[l,e1]  (bf16, M=1; the
                # be2-dependent constant offset cancels in the softmax)
                psc = ps_sm.tile([1, T], dt.float32, tag="sm")
                for ec in range(EC):
                    nc.tensor.matmul(
                        psc,
                        lhsT=vsc_sb[:, l, ec : ec + 1],
                        rhs=x1t[:, ec, :],
                        start=(ec == 0),
                        stop=(ec == EC - 1),
                    )

                # softmax weights (normalized) on [1, T]
                negmax = smalls.tile([1, 1], dt.float32, tag="negmax")
                nc.vector.tensor_reduce(
                    out=negmax, in_=psc, axis=AX.X, op=ALU.max, negate=True
                )
                ew = smalls.tile([1, T], dt.float32, tag="ew")
                ewsum = smalls.tile([1, 1], dt.float32, tag="ewsum")
                nc.scalar.activation(
                    out=ew, in_=psc, func=AF.Exp, bias=negmax, accum_out=ewsum
                )
                rsum = smalls.tile([1, 1], dt.float32, tag="rsum")
                nc.vector.reciprocal(out=rsum, in_=ewsum)
                ewn = smalls.tile([1, T], dt.float32, tag="ewn")
                nc.vector.tensor_scalar_mul(ewn, ew, rsum)
                ewb = xpool.tile([128, T], dt.float32, tag="ewb")
                nc.gpsimd.partition_broadcast(ewb, ewn)

                # pooled[e] = sum_t x2T[e,t] * ewb[t]
                # (tensor_tensor_reduce hard-crashes TRN2 here, so mul+reduce)
                scr = xpool.tile([128, EC, T], dt.float32, tag="scr")
                for fc in range(EC):
                    nc.vector.tensor_tensor(
                        out=scr[:, fc, :], in0=x2t[:, fc, :], in1=ewb, op=ALU.mult
                    )
                nc.vector.reduce_sum(
                    out=pooled_sb[:, l, :, b : b + 1], in_=scr, axis=AX.X
                )

        # ---------------- epilogue: pooled output + router + head ----------------
        # transpose pooled to natural layout [l, b, e]
        for b in range(BL):
            for fc in range(EC):
                pbt = ps_sm.tile([L, 128], dt.float32, tag="sm")
                nc.tensor.matmul(
                    pbt,
                    lhsT=pooled_sb[:, :, fc, b],
                    rhs=id128,
                    is_transpose=True,
                    start=True,
                    stop=True,
                )
                nc.scalar.copy(out=pooled_t[:, b, fc * 128 : (fc + 1) * 128], in_=pbt)
            nc.sync.dma_start(out=o_pooled[b], in_=pooled_t[:, b, :])

        # router input in bf16
        ri_bf = singles.tile([128, L, EC, BL], dt.bfloat16, tag="ribf")
        nc.vector.tensor_copy(ri_bf, pooled_sb)

        # rhT[128,b] = relu(Wr1^T @ ri + br1)
        prh = ps_sm.tile([RH, BL], dt.float32, tag="sm")
        n = 0
        for l in range(L):
            for fc in range(EC):
                nc.tensor.matmul(
                    prh,
                    lhsT=wr1_sb[:, l, fc, :],
                    rhs=ri_bf[:, l, fc, :],
                    start=(n == 0),
                    stop=(n == L * EC - 1),
                )
                n += 1
        rh_sb = epi.tile([RH, BL], dt.bfloat16, tag="rh")
        nc.scalar.activation(out=rh_sb, in_=prh, func=AF.Relu, bias=br1_sb)

        # noise_scale = softplus(Wn^T @ ri + bn)  [l, b]
        pns = ps_sm.tile([L, BL], dt.float32, tag="sm")
        n = 0
        for l in range(L):
            for fc in range(EC):
                nc.tensor.matmul(
                    pns,
                    lhsT=wn_sb[:, l, fc, :],
                    rhs=ri_bf[:, l, fc, :],
                    start=(n == 0),
                    stop=(n == L * EC - 1),
                )
                n += 1
        ns_sb = epi.tile([L, BL], dt.float32, tag="ns")
        nse = epi.tile([L, BL], dt.float32, tag="nse")
        nc.scalar.activation(out=nse, in_=pns, func=AF.Exp, bias=bn_sb)
        nc.scalar.activation(out=ns_sb, in_=nse, func=AF.Ln, bias=1.0)

        # logits_r^T [l, b]
        plr = ps_sm.tile([L, BL], dt.float32, tag="sm")
        nc.tensor.matmul(plr, lhsT=wr2_sb, rhs=rh_sb, start=True, stop=True)

        # learned = sigmoid((lr + br2 + noise*ns)/TEMP); rw = A*learned + (1-A)/L
        nst = epi.tile([L, BL], dt.float32, tag="nst")
        nc.vector.tensor_tensor(out=nst, in0=noise_sb, in1=ns_sb, op=ALU.mult)
        s1 = epi.tile([L, BL], dt.float32, tag="s1")
        nc.vector.scalar_tensor_tensor(
            out=s1, in0=plr, scalar=br2_sb, in1=nst, op0=ALU.add, op1=ALU.add
        )
        rwt = epi.tile([L, BL], dt.float32, tag="rwt")
        nc.scalar.activation(out=rwt, in_=s1, func=AF.Sigmoid, scale=1.0 / TEMP)
        nc.scalar.activation(
            out=rwt, in_=rwt, func=AF.Copy, scale=ALPHA, bias=(1.0 - ALPHA) / L
        )

        # rw natural [b, l] -> output
        prwn = ps_sm.tile([BL, L], dt.float32, tag="sm")
        nc.tensor.matmul(
            prwn, lhsT=rwt, rhs=id128[0:L, 0:L], is_transpose=True, start=True, stop=True
        )
        rw_sb = epi.tile([BL, L], dt.float32, tag="rwn")
        nc.scalar.copy(out=rw_sb, in_=prwn)
        nc.sync.dma_start(out=o_rw, in_=rw_sb)

        # fused[b, e] = sum_l pooled[l, b, e] * rw[l, b]   (bf16 matvec),
        # then LayerNorm per b on [1, E] rows (engine APs must start at an
        # aligned partition, so b=1 cannot live on partition 1).
        rwt_bf = epi.tile([L, BL], dt.bfloat16, tag="rwtbf")
        nc.scalar.copy(out=rwt_bf, in_=rwt)
        pooled_bf = epi.tile([L, BL, E], dt.bfloat16, tag="pooledbf")
        nc.vector.tensor_copy(pooled_bf, pooled_t)
        fnT = epi.tile([128, EC, BL], dt.bfloat16, tag="fnT")
        for b in range(BL):
            pf = ps_sm.tile([1, E], dt.float32, tag="sm")
            nc.tensor.matmul(
                pf,
                lhsT=rwt_bf[:, b : b + 1],
                rhs=pooled_bf[:, b, :],
                start=True,
                stop=True,
            )
            f2 = epi.tile([1, E], dt.float32, tag=f"fused{b}")
            nc.scalar.copy(out=f2, in_=pf)
            mu = epi.tile([1, 1], dt.float32, tag=f"mu{b}")
            nc.vector.reduce_sum(out=mu, in_=f2, axis=AX.X)
            nc.scalar.mul(mu, mu, 1.0 / E)
            cen = epi.tile([1, E], dt.float32, tag=f"cen{b}")
            nc.vector.tensor_scalar(
                out=cen, in0=f2, scalar1=mu, scalar2=None, op0=ALU.subtract
            )
            var = epi.tile([1, 1], dt.float32, tag=f"var{b}")
            vscr = epi.tile([1, E], dt.float32, tag=f"vscr{b}")
            nc.vector.tensor_tensor(out=vscr, in0=cen, in1=cen, op=ALU.mult)
            nc.vector.reduce_sum(out=var, in_=vscr, axis=AX.X)
            sd = epi.tile([1, 1], dt.float32, tag=f"sd{b}")
            # sd = sqrt(sum(cen^2)/E + eps)
            nc.scalar.activation(
                out=sd, in_=var, func=AF.Sqrt, bias=eps_sb, scale=1.0 / E
            )
            rstd = epi.tile([1, 1], dt.float32, tag=f"rstd{b}")
            nc.vector.reciprocal(out=rstd, in_=sd)
            normed = epi.tile([1, E], dt.float32, tag=f"normed{b}")
            nc.vector.tensor_scalar_mul(normed, cen, rstd)
            fn_sb = epi.tile([1, E], dt.float32, tag=f"fn{b}")
            nc.vector.tensor_tensor(out=fn_sb, in0=normed, in1=lng_sb, op=ALU.mult)
            nc.vector.tensor_tensor(out=fn_sb, in0=fn_sb, in1=lnb_sb, op=ALU.add)
            nc.sync.dma_start(out=o_fn[b], in_=fn_sb)

            # transpose fn row into fnT[:, :, b] for the classifier
            for c in range(EC):
                pft = ps_sm.tile([128, 1], dt.float32, tag="sm")
                nc.tensor.matmul(
                    pft,
                    lhsT=fn_sb[:, c * 128 : (c + 1) * 128],
                    rhs=id128[0:1, 0:1],
                    is_transpose=True,
                    start=True,
                    stop=True,
                )
                nc.scalar.copy(out=fnT[:, c, b : b + 1], in_=pft)
        h1t = epi.tile([128, 2, BL], dt.bfloat16, tag="h1t")
        for cc in range(2):
            ph1 = ps_sm.tile([128, BL], dt.float32, tag="sm")
            for ec in range(EC):
                nc.tensor.matmul(
                    ph1,
                    lhsT=wc1_sb[:, ec, cc * 128 : (cc + 1) * 128],
                    rhs=fnT[:, ec, :],
                    start=(ec == 0),
                    stop=(ec == EC - 1),
                )
            nc.scalar.activation(
                out=h1t[:, cc, :], in_=ph1, func=AF.Relu, bias=bc1_sb[:, cc : cc + 1]
            )
        plg = ps_sm.tile([NCLS, BL], dt.float32, tag="sm")
        for cc in range(2):
            nc.tensor.matmul(
                plg,
                lhsT=wc2_sb[:, cc, :],
                rhs=h1t[:, cc, :],
                start=(cc == 0),
                stop=(cc == 1),
            )
        lg_sb = epi.tile([NCLS, BL], dt.float32, tag="lg")
        nc.vector.tensor_scalar_add(lg_sb, plg, bc2_sb)
        nc.sync.dma_start(out=o_logits.rearrange("b c -> c b"), in_=lg_sb)

    nc.compile()
    return nc


def _get_program():
    global _PROGRAM
    if _PROGRAM is None:
        _PROGRAM = _build_program()
    return _PROGRAM


def make_in_maps(h, noise, We1, be1, We2, be2, wa, ba, Wr1, br1, Wr2, br2,
                 Wn, bn, ln_g, ln_b, Wc1, bc1, Wc2, bc2):
    bf = ml_dtypes.bfloat16
    f32 = np.float32
    shared = {
        "we1": np.ascontiguousarray(We1, f32).astype(bf),
        "we2": np.ascontiguousarray(We2, f32).astype(bf),
        "vsc": np.ascontiguousarray(
            np.einsum("lef,f->le", np.asarray(We2, np.float64), np.asarray(wa, np.float64))
        ).astype(bf),
        "wr1": np.ascontiguousarray(Wr1, f32).astype(bf),
        "wn": np.ascontiguousarray(Wn, f32).astype(bf),
        "wr2": np.ascontiguousarray(Wr2, f32).astype(bf),
        "wc1": np.ascontiguousarray(Wc1, f32).astype(bf),
        "wc2": np.ascontiguousarray(Wc2, f32).astype(bf),
        "be1t": np.ascontiguousarray(
            np.asarray(be1, f32).reshape(L, EC, 128).transpose(2, 0, 1).reshape(128, L * EC)
        ),
        "be2t": np.ascontiguousarray(
            np.asarray(be2, f32).reshape(L, EC, 128).transpose(2, 0, 1).reshape(128, L * EC)
        ),
        "br1c": np.ascontiguousarray(np.asarray(br1, f32).reshape(RH, 1)),
        "br2c": np.ascontiguousarray(np.asarray(br2, f32).reshape(L, 1)),
        "bnc": np.ascontiguousarray(np.asarray(bn, f32).reshape(L, 1)),
        "bc1t": np.ascontiguousarray(np.asarray(bc1, f32).reshape(2, 128).T),
        "bc2c": np.ascontiguousarray(np.asarray(bc2, f32).reshape(NCLS, 1)),
        "lng": np.ascontiguousarray(np.asarray(ln_g, f32).reshape(1, E)),
        "lnb": np.ascontiguousarray(np.asarray(ln_b, f32).reshape(1, E)),
    }
    h = np.asarray(h, f32)
    noise = np.asarray(noise, f32)
    in_maps = []
    for c in range(N_CORES):
        m = dict(shared)
        m["hx"] = np.ascontiguousarray(h[c * BL : (c + 1) * BL])
        m["noiset"] = np.ascontiguousarray(noise[c * BL : (c + 1) * BL].T)
        in_maps.append(m)
    return in_maps


def kernel(**inputs):
    from concourse.bass_utils import run_bass_kernel_spmd

    nc = _get_program()
    in_maps = make_in_maps(**inputs)
    res = run_bass_kernel_spmd(nc, in_maps, core_ids=list(range(N_CORES)))
    logits = np.concatenate([r["o_logits"] for r in res.results], axis=0)
    rw = np.concatenate([r["o_rw"] for r in res.results], axis=0)
    fn = np.concatenate([r["o_fn"] for r in res.results], axis=0)
    pooled = np.concatenate([r["o_pooled"] for r in res.results], axis=0)
    return logits, rw, fn, pooled
